# revision 27
# baseline (speedup 1.0000x reference)
"""Trainium2 Bass kernel for a 3-layer binary-weight MLP.

Problem (nn_MLP_56779467653689):
    x: [8192, 1024] f32
    h = relu(s0 * (x @ W0)) * 2      W0 = 2*k0-1  in {-1,+1}, [1024, 4096]
    h = relu(s1 * (h @ W1)) * 2      W1 [4096, 4096]
    out = s2 * (h @ W2)              W2 [4096, 1024]

Strategy: pure data-parallel over tokens across 8 NeuronCores (1024
tokens/core). Per core, activations live in SBUF as [features, tokens]
(features on partitions) so layers chain with no transposes. Weights are
pre-packed on the host into per-output-strip SBUF layout and streamed from
HBM.

Precision plan: layer 0 runs in bf16 (±1 weights exact, fp32 PSUM). Layer
1 — 2/3 of the FLOPs — and the first J2/32 contraction tiles of layer 2
run in fp8e4m3 with perf_mode=DoubleRow, which packs two contraction rows
per PE cell: one matmul contracts a 256-row k-tile pair at the same
~216 ns issue rate as a bf16 matmul, i.e. 2x the FLOP rate. ±1 weights
are exact in fp8; only the activations quantize. To shrink that error the
activations are centered before the cast (q = e4m3(h - C)) and the exact
correction C*colsum(W) is added back via the per-partition bias operand
of the eviction ACT (relu(a*(q@W) + a*C*colsum) == next h) or of the
output eviction. C = 2.0 is exactly representable in e4m3, so the ~50% of
relu outputs that are exactly zero quantize with no error; the sweep in
sweep2.py picks C and J2. Measured: rel_err 1.88e-2 (gate 2e-2), HW
446.8 us at full clock vs 687.6 us for the all-bf16 baseline
(kernel_bf16_baseline.py). The device intermittently clock-gates the PE
to 5/6 rate (259 ns/matmul, whole-NEFF, also visible on pure-bf16 runs as
~825 us); timings above are from unthrottled windows.
"""

from contextlib import ExitStack

import ml_dtypes
import numpy as np

P = 128
TOKENS = 8192
D_IN = 1024
D_H = 4096
D_OUT = 1024
N_CORES = 8
TOK_PER_CORE = TOKENS // N_CORES  # 1024
TOK_TILE = 512
NT = TOK_PER_CORE // TOK_TILE  # 2
KT1 = D_H // P  # 32 contraction tiles for layer 1

# Centering offset for the fp8 quantization of h1 (see module docstring).
# Exactly representable in e4m3 so the 50% of h1 entries that are exactly
# zero quantize with no error.
C1 = 2.0

# Layer 2 runs its first J2 contraction tiles in fp8 DoubleRow too (h2
# centered by C2, corrected via colsum bias), the rest in bf16. J2 = 12
# spends the remaining error budget (sim: 1.92e-2 total, measured HW runs
# consistently ~0.04 below sim) for 96 fewer matmuls than all-bf16.
J2 = 12
C2 = 2.0

BF16 = ml_dtypes.bfloat16
F8 = ml_dtypes.float8_e4m3

# Set TRACE=True (from test.py) to profile; LAST_EXEC_TIME_NS then holds the
# max per-core HW exec time of the most recent kernel() call.
TRACE = False
TRACE_CORES = None  # e.g. list(range(8)) to profile every core
LAST_EXEC_TIME_NS = None
LAST_RESULT = None

_cache = {}


def _dense_layer(nc, wpool, pspool, in_slice, w_dram, k_sub, n_t, evict,
                 t_outer=False):
    """out[n] strips = act(W[:, n-strip].T @ in) for n in range(n_t).

    in_slice(j, t): AP of the input block [P, TOK_TILE] for contraction
    tile j, token tile t. w_dram: packed [n_t, P, k_sub*P]. evict(n, t, ps)
    consumes the accumulated PSUM tile for (output strip n, token tile t).
    t_outer: each accumulation chain touches one token half, so layer 1's
    first chain starts after only the t=0 input halves landed.
    """
    import concourse.mybir as mybir

    for n in range(n_t):
        w = wpool.tile([P, k_sub * P], mybir.dt.bfloat16, tag="w", name=f"w_{n}")
        if n == 0 and t_outer:
            # Split the first strip so the first accumulation chain's k-tile
            # lands (and the first matmul issues) before the rest streams in.
            nc.sync.dma_start(out=w[:, : 2 * P], in_=w_dram[0][:, : 2 * P])
            nc.sync.dma_start(out=w[:, 2 * P :], in_=w_dram[0][:, 2 * P :])
        else:
            nc.sync.dma_start(out=w[:], in_=w_dram[n])
        if t_outer:
            for t in range(NT):
                ps = pspool.tile(
                    [P, TOK_TILE], mybir.dt.float32, tag="ps", name=f"ps_{n}_{t}"
                )
                for j in range(k_sub):
                    nc.tensor.matmul(
                        ps[:],
                        w[:, j * P : (j + 1) * P],
                        in_slice(j, t),
                        start=(j == 0),
                        stop=(j == k_sub - 1),
                    )
                evict(n, t, ps)
        else:
            # t-inner: consecutive matmuls alternate PSUM banks, which
            # measures ~0.7 ns/MM faster than same-bank accumulation runs.
            pss = [
                pspool.tile(
                    [P, TOK_TILE], mybir.dt.float32, tag="ps", name=f"ps_{n}_{t}"
                )
                for t in range(NT)
            ]
            for j in range(k_sub):
                for t in range(NT):
                    nc.tensor.matmul(
                        pss[t][:],
                        w[:, j * P : (j + 1) * P],
                        in_slice(j, t),
                        start=(j == 0),
                        stop=(j == k_sub - 1),
                    )
            for t in range(NT):
                evict(n, t, pss[t])


def _prune_dma_waits(nc, max_waits=1):
    """Drop transitively-implied waits from DMA instructions.

    DMA queue-entry descriptors hold a single sync wait; Tile's sem
    assignment is per-proc minimal but not transitively minimal across
    procs, so a recycled SBUF slot's DMA can carry WAR (engine) + WAW
    (prev slot writer's DMA lane) + lane-recycle waits = 3. The WAW (and
    often the recycle) wait is implied by the engine wait: the readers
    counted by the WAR threshold themselves waited on those DMAs.

    Soundness: a wait (s >= v) on instruction I is dropped only when the
    completion clocks implied by I's *other* waits already guarantee
    cumulative increments of s reached v. Completion clocks are built
    forward over the scheduled BIR order giving same-stream predecessor
    credit only to in-order engines (PE/ACT/DVE/SP), never to DMA lanes
    or Pool. Unrecognized wait/update modes contribute no credit, so
    unknowns can only inhibit pruning, never enable it.
    """
    import bisect

    import bass_rust

    IN_ORDER_ENGINES = {
        "EngineType.PE",
        "EngineType.Activation",
        "EngineType.DVE",
        "EngineType.SP",
    }

    sem_hist = {}  # sem -> ([cumulative values], [clocks at completion])
    sem_cum = {}  # sem -> cumulative increments so far
    eng_clock = {}  # engine -> completion clock of last instruction
    poisoned = set()  # sems with non-monotonic updates: no credit

    def cc(sem, val):
        """Completion clock implied by observing sem >= val, or None."""
        if sem in poisoned:
            return None
        hist = sem_hist.get(sem)
        if not hist or hist[0][-1] < val:
            return None
        return hist[1][bisect.bisect_left(hist[0], val)]

    def merge(dst, src):
        for k, v in src.items():
            if dst.get(k, 0) < v:
                dst[k] = v

    pruned = 0
    for bb in nc.m.functions[0].blocks:
        for inst in bb.instructions:
            si = inst.sync_info
            waits = list(si.on_wait or []) if si is not None else []
            ups = list(si.on_update or []) if si is not None else []
            is_dma = type(inst).__name__ == "InstDMACopy"

            clock = {}
            if not is_dma:
                prev = eng_clock.get(str(inst.engine))
                if prev is not None and str(inst.engine) in IN_ORDER_ENGINES:
                    merge(clock, prev)
            for w in waits:
                if w.wait_mode == "sem-ge-imm" and w.wait_value is not None:
                    c = cc(w.ant_name, w.wait_value)
                    if c is not None:
                        merge(clock, c)

            # Per-encoding wait budgets: DMA queue entries hold 1 wait;
            # engine instructions hold 2. Drain/EventSemaphore/control flow
            # are lowered specially by walrus — leave them alone.
            tname = type(inst).__name__
            if is_dma:
                cap = max_waits
            elif tname in ("InstDrain", "InstEventSemaphore", "InstCall",
                           "InstUnconditionalBranch", "InstISA"):
                cap = None
            else:
                cap = 2

            if cap is not None and len(waits) > cap:
                kept = list(waits)
                changed = True
                while len(kept) > cap and changed:
                    changed = False
                    for w in list(kept):
                        if w.wait_mode != "sem-ge-imm" or w.wait_value is None:
                            continue
                        implied = {}
                        provable = True
                        for o in kept:
                            if o is w:
                                continue
                            if o.wait_mode != "sem-ge-imm" or o.wait_value is None:
                                provable = False
                                break
                            c = cc(o.ant_name, o.wait_value)
                            if c is None:
                                provable = False
                                break
                            merge(implied, c)
                        if provable and implied.get(w.ant_name, 0) >= w.wait_value:
                            kept.remove(w)
                            pruned += 1
                            changed = True
                            break
                # Anything still over budget is left for Bacc's
                # generate_event_semaphores pass to split legally.
                if len(kept) != len(waits):
                    inst.sync_info = bass_rust.SyncInfo(on_wait=kept, on_update=ups)

            own = {}
            for u in ups:
                if u.update_mode not in ("sem-inc", "sem-add-imm"):
                    poisoned.add(u.ant_name)
                    continue
                inc = 1 if u.update_mode == "sem-inc" else u.update_value
                if inc is None:
                    poisoned.add(u.ant_name)
                    continue
                sem = u.ant_name
                sem_cum[sem] = sem_cum.get(sem, 0) + inc
                own[sem] = sem_cum[sem]
            merge(clock, own)
            for sem, cum in own.items():
                vals, clocks = sem_hist.setdefault(sem, ([], []))
                vals.append(cum)
                clocks.append(clock)
            if not is_dma:
                eng_clock[str(inst.engine)] = clock
    return pruned


def _build(a0, a1, a2):
    """Build the SPMD single-core program (same NEFF on all 8 cores)."""
    import concourse.mybir as mybir
    import concourse.tile as tile
    from concourse import bacc

    # Bacc (not plain Bass): its finalize() runs the wait-legalization
    # passes (move_matmul_waits_to_ldweights, generate_event_semaphores)
    # that split multi-wait instructions to the 1-wait HW encoding.
    nc = bacc.Bacc(
        "TRN2",
        target_bir_lowering=False,
        debug=False,
        enable_asserts=False,
        num_devices=N_CORES,
    )
    bf = mybir.dt.bfloat16
    f32 = mybir.dt.float32
    f8 = mybir.dt.float8e4

    xt = nc.dram_tensor("xt", [D_IN, TOK_PER_CORE], bf, kind="ExternalInput")
    w0p = nc.dram_tensor("w0p", [D_H // P, P, D_IN], bf, kind="ExternalInput")
    w1p = nc.dram_tensor("w1p", [KT1, P, D_H], f8, kind="ExternalInput")
    w2q = nc.dram_tensor("w2q", [D_OUT // P, P, J2 * P], f8, kind="ExternalInput")
    w2b = nc.dram_tensor(
        "w2b", [D_OUT // P, P, (KT1 - J2) * P], bf, kind="ExternalInput"
    )
    # b1[p, n] = a1 * C1 * colsum(W1)[n*P + p]: the exact correction for the
    # centered-fp8 h1, folded into layer 1's eviction ACT as per-partition bias.
    b1d = nc.dram_tensor("b1", [P, KT1], f32, kind="ExternalInput")
    # b2: same correction for the centered-fp8 first-J2*P rows of layer 2.
    b2d = nc.dram_tensor("b2", [P, D_OUT // P], f32, kind="ExternalInput")
    outt = nc.dram_tensor("outt", [D_OUT, TOK_PER_CORE], f32, kind="ExternalOutput")

    relu = mybir.ActivationFunctionType.Relu
    dr = mybir.MatmulPerfMode.DoubleRow

    with tile.TileContext(nc) as tc, ExitStack() as ctx:
        xpool = ctx.enter_context(tc.tile_pool(name="xp", bufs=1))
        q1pool = ctx.enter_context(tc.tile_pool(name="q1p", bufs=1))
        h2pool = ctx.enter_context(tc.tile_pool(name="h2p", bufs=1))
        wpool = ctx.enter_context(tc.tile_pool(name="wp", bufs=4))
        tpool = ctx.enter_context(tc.tile_pool(name="tp", bufs=4))
        bpool = ctx.enter_context(tc.tile_pool(name="bp", bufs=1))
        opool = ctx.enter_context(tc.tile_pool(name="op", bufs=3))
        pspool = ctx.enter_context(tc.tile_pool(name="psp", bufs=8, space="PSUM"))

        # x as per-j half-tiles in consumption order (t=0 first): the first
        # accumulation chain starts after just x_0_0 (128 KB) + one weight
        # strip, with later tiles streaming in behind the compute. Spread
        # across the ACT/DVE/GpSimd HWDGE queues (weights own the SP queue)
        # so all of x lands ~3x sooner and the t=1 chains never stall.
        xq = [nc.scalar, nc.gpsimd]
        x_half = [[None] * NT for _ in range(D_IN // P)]
        for t in range(NT):
            for j in range(D_IN // P):
                h = xpool.tile([P, TOK_TILE], bf, tag=f"x{j}_{t}", name=f"x_{j}_{t}")
                src = xt[j * P : (j + 1) * P, t * TOK_TILE : (t + 1) * TOK_TILE]
                if t == 0 and j == 0:
                    # The very first tile gates the first matmul; halve its
                    # transfer latency by splitting its partition rows (the
                    # DMA descriptor axis) across both queues.
                    nc.scalar.dma_start(out=h[: P // 2, :], in_=src[: P // 2, :])
                    nc.gpsimd.dma_start(out=h[P // 2 :, :], in_=src[P // 2 :, :])
                else:
                    xq[(t * (D_IN // P) + j) % len(xq)].dma_start(out=h[:], in_=src)
                x_half[j][t] = h

        def x_slice(j, t):
            return x_half[j][t][:]

        # Layer-1 input: centered-fp8 h1, stored [p, j, tok] so a DoubleRow
        # matmul reads k-tile pairs as the 3D AP [:, 2m:2m+2, t*512:+512].
        q1 = q1pool.tile([P, KT1, TOK_PER_CORE], f8, tag="q1", name="q1")
        # Layer-2 input: first J2 k-tiles centered-fp8, the rest bf16.
        q2 = q1pool.tile([P, J2, TOK_PER_CORE], f8, tag="q2", name="q2")
        h2_tiles = [
            h2pool.tile([P, TOK_PER_CORE], bf, tag=f"h2_{n}", name=f"h2_{n}")
            if n >= J2
            else None
            for n in range(D_H // P)
        ]

        def evict_q1(n, t, ps):
            # relu + scale on ACT (PSUM -> bf16), center + downcast on DVE.
            hh = tpool.tile([P, TOK_TILE], bf, tag="h1t", name=f"h1t_{n}_{t}")
            nc.scalar.activation(hh[:], ps[:], relu, scale=a0)
            nc.vector.tensor_scalar_sub(
                q1[:, n : n + 1, t * TOK_TILE : (t + 1) * TOK_TILE], hh[:], C1
            )

        def evict_out(n, t, ps, chunks=1):
            # out = a2*acc + b2[n-strip] (the centered-fp8 correction for the
            # q2 part of the contraction). The two token halves evict on
            # different engines (ACT / DVE) and DMA on different queues so
            # the last strip's evictions run fully in parallel. The final
            # strip additionally evicts in 256-token chunks (chunks=2) so
            # its output DMAs start sooner and the kernel tail shrinks.
            cw = TOK_TILE // chunks
            for c in range(chunks):
                o = opool.tile([P, cw], f32, tag=f"o{c}", name=f"o_{n}_{t}_{c}")
                if t % 2 == 0:
                    nc.scalar.activation(
                        o[:],
                        ps[:, c * cw : (c + 1) * cw],
                        mybir.ActivationFunctionType.Identity,
                        bias=b2[:, n : n + 1],
                        scale=a2,
                    )
                else:
                    nc.vector.tensor_scalar(
                        o[:],
                        ps[:, c * cw : (c + 1) * cw],
                        a2,
                        b2[:, n : n + 1],
                        op0=mybir.AluOpType.mult,
                        op1=mybir.AluOpType.add,
                    )
                base = t * TOK_TILE + c * cw
                (nc.scalar if t % 2 == 0 else nc.gpsimd).dma_start(
                    out=outt[n * P : (n + 1) * P, base : base + cw],
                    in_=o[:],
                )

        # Bias vectors load behind layer 0's weight stream — b1 is first
        # needed at layer 1's evictions, b2 at layer 2's. Issuing them here
        # keeps the first w0 strip at the head of the SP DMA queue, which
        # gates the first matmul.
        b1 = bpool.tile([P, KT1], f32, tag="b1", name="b1")
        b2 = bpool.tile([P, D_OUT // P], f32, tag="b2", name="b2")
        nc.scalar.dma_start(out=b1[:], in_=b1d[:, :])
        nc.scalar.dma_start(out=b2[:], in_=b2d[:, :])

        _dense_layer(nc, wpool, pspool, x_slice, w0p,
                     D_IN // P, D_H // P, evict_q1, t_outer=True)

        # Layer 1: fp8 DoubleRow. Each matmul contracts a 256-row k-tile
        # pair (weights [128, 2, 128], moving [128, 2, 512]) at ~1 moving
        # pair per PE cycle — 2x the bf16 FLOP rate.
        for n in range(D_H // P):
            w = wpool.tile([P, KT1, P], f8, tag="w", name=f"w1_{n}")
            nc.sync.dma_start(out=w[:], in_=w1p[n])
            pss = [
                pspool.tile([P, TOK_TILE], f32, tag="ps", name=f"ps1_{n}_{t}")
                for t in range(NT)
            ]
            for m in range(KT1 // 2):
                for t in range(NT):
                    nc.tensor.matmul(
                        pss[t][:],
                        w[:, 2 * m : 2 * m + 2, :],
                        q1[:, 2 * m : 2 * m + 2, t * TOK_TILE : (t + 1) * TOK_TILE],
                        start=(m == 0),
                        stop=(m == KT1 // 2 - 1),
                        perf_mode=dr,
                    )
            for t in range(NT):
                if n < J2:
                    hh = tpool.tile([P, TOK_TILE], bf, tag="h2t", name=f"h2t_{n}_{t}")
                    nc.scalar.activation(
                        hh[:], pss[t][:], relu, bias=b1[:, n : n + 1], scale=a1
                    )
                    nc.vector.tensor_scalar_sub(
                        q2[:, n : n + 1, t * TOK_TILE : (t + 1) * TOK_TILE], hh[:], C2
                    )
                else:
                    nc.scalar.activation(
                        h2_tiles[n][:, t * TOK_TILE : (t + 1) * TOK_TILE],
                        pss[t][:],
                        relu,
                        bias=b1[:, n : n + 1],
                        scale=a1,
                    )

        # Layer 2: first J2 k-tiles as fp8 DoubleRow (q2), rest bf16 (h2).
        # One PSUM accumulation chain mixes both matmul modes.
        for n in range(D_OUT // P):
            wq = wpool.tile([P, J2, P], f8, tag="w2q", name=f"w2q_{n}")
            nc.sync.dma_start(out=wq[:], in_=w2q[n])
            wb = wpool.tile([P, (KT1 - J2) * P], bf, tag="w2b", name=f"w2b_{n}")
            nc.sync.dma_start(out=wb[:], in_=w2b[n])
            pss = [
                pspool.tile([P, TOK_TILE], f32, tag="ps", name=f"ps2_{n}_{t}")
                for t in range(NT)
            ]
            for m in range(J2 // 2):
                for t in range(NT):
                    nc.tensor.matmul(
                        pss[t][:],
                        wq[:, 2 * m : 2 * m + 2, :],
                        q2[:, 2 * m : 2 * m + 2, t * TOK_TILE : (t + 1) * TOK_TILE],
                        start=(m == 0),
                        stop=False,
                        perf_mode=dr,
                    )
            for j in range(KT1 - J2):
                for t in range(NT):
                    nc.tensor.matmul(
                        pss[t][:],
                        wb[:, j * P : (j + 1) * P],
                        h2_tiles[J2 + j][:, t * TOK_TILE : (t + 1) * TOK_TILE],
                        start=False,
                        stop=(j == KT1 - J2 - 1),
                    )
            for t in range(NT):
                evict_out(n, t, pss[t], chunks=2 if n == D_OUT // P - 1 else 1)

    _prune_dma_waits(nc)
    nc.finalize()
    return nc


def _pack_w(k, dtype=BF16):
    """Bool [K, N] -> ±1 packed [N/P, P, K]: strip n, partition p,
    free j*P+c  <-  W[j*P+p, n*P+c] (partition = contraction for lhsT)."""
    K, N = k.shape
    w = np.where(k, np.float32(1.0), np.float32(-1.0)).astype(dtype)
    return np.ascontiguousarray(
        w.reshape(K // P, P, N // P, P).transpose(2, 1, 0, 3).reshape(N // P, P, K)
    )


def _enable_ntff_trace():
    """Best-effort plumbing for trace=True under axon in this image.

    The image's ``antenv`` lacks the ``axon_hooks`` shim that
    ``trn_agent_boot`` would normally register the NTFF profile hook
    into, and there is no artifact bucket — stub both.
    """
    import sys
    import types

    import concourse.bass_utils as bu

    bu.upload_artifacts = lambda tmpdir: tmpdir
    try:
        from antenv import axon_hooks
    except ImportError:
        import antenv

        axon_hooks = types.ModuleType("antenv.axon_hooks")
        _state = {"hook": None}
        axon_hooks.set_axon_ntff_profile_hook = lambda h: _state.__setitem__(
            "hook", h
        )
        axon_hooks.get_axon_ntff_profile_hook = lambda: _state["hook"]
        sys.modules["antenv.axon_hooks"] = axon_hooks
        antenv.axon_hooks = axon_hooks
    if axon_hooks.get_axon_ntff_profile_hook() is None:
        from trn_agent_boot.trn_boot import _ntff_profile_via_ctypes

        axon_hooks.set_axon_ntff_profile_hook(
            _ntff_profile_via_ctypes("/opt/axon/libaxon_pjrt.so")
        )


def kernel(x, k0, k1, k2, s0, s1, s2):
    global LAST_EXEC_TIME_NS, LAST_RESULT
    from concourse.bass_utils import run_bass_kernel_spmd

    if TRACE:
        _enable_ntff_trace()

    x = np.asarray(x)
    a0 = 2.0 * float(np.asarray(s0))
    a1 = 2.0 * float(np.asarray(s1))
    a2 = float(np.asarray(s2))

    key = (a0, a1, a2)
    if key not in _cache:
        _cache[key] = _build(a0, a1, a2)
    nc = _cache[key]

    w0p = _pack_w(np.asarray(k0))
    w1p = _pack_w(np.asarray(k1), F8)
    w2full = _pack_w(np.asarray(k2))
    w2q = np.ascontiguousarray(w2full[:, :, : J2 * P]).astype(F8)
    w2b = np.ascontiguousarray(w2full[:, :, J2 * P :])
    colsum1 = 2.0 * np.asarray(k1).sum(axis=0, dtype=np.int64) - D_H
    b1 = np.ascontiguousarray(
        (a1 * C1 * colsum1).astype(np.float32).reshape(KT1, P).T
    )
    colsum2q = 2.0 * np.asarray(k2)[: J2 * P].sum(axis=0, dtype=np.int64) - J2 * P
    b2 = np.ascontiguousarray(
        (a2 * C2 * colsum2q).astype(np.float32).reshape(D_OUT // P, P).T
    )

    in_maps = []
    for i in range(N_CORES):
        xs = x[i * TOK_PER_CORE : (i + 1) * TOK_PER_CORE].astype(BF16)
        in_maps.append(
            {
                "xt": np.ascontiguousarray(xs.T),
                "w0p": w0p,
                "w1p": w1p,
                "w2q": w2q,
                "w2b": w2b,
                "b1": b1,
                "b2": b2,
            }
        )

    res = run_bass_kernel_spmd(
        nc, in_maps, list(range(N_CORES)), trace=TRACE, trace_cores=TRACE_CORES
    )
    LAST_EXEC_TIME_NS = res.exec_time_ns
    LAST_RESULT = res
    out = np.concatenate(
        [res.results[i]["outt"].T for i in range(N_CORES)], axis=0
    )
    return np.ascontiguousarray(out)

